# revision 12
# baseline (speedup 1.0000x reference)
"""Multi-head causal attention (B=2, S=2048, D=1024, H=16, hd=64) on 8 trn2 cores.

Sharding: core c handles batch b = c//4 and head-group g = c%4 (heads 4g..4g+4,
d-slice 256g..256g+256 of the QKV projections / Wo rows).  Each core computes a
partial out-projection [2048, 1024]; the host sums the 4 head-group partials per
batch and adds the bias.

Per-core kernel (all matmuls bf16, accumulate f32 in PSUM):
  qT/kT = (x @ Wq/k)^T computed directly as [256, 2048] via lhsT=W chunks.
  v     = x @ Wv in natural [seq, head, 64(+1 ones col)] layout (ones column
          makes the attention rowsum fall out of the ctx matmul).
  S^T   = k_h @ q_h^T  [kpos, qpos] tiles; exp via ACT (scale=1/8) PSUM->SBUF;
          causal handled by skipping invalid column blocks + one triangular
          bf16 mask multiply on diagonal blocks.
  ctx~T = v'_h^T @ expS^T accumulated over kpos blocks -> [65, 512] PSUM
          (row 64 = softmax denominator).
  out  += (ctx~T / rowsum)^T @ Wo rows  (normalization via DVE multiply with a
          GPSIMD partition-broadcast reciprocal).
"""

import sys

import numpy as np

for _p in ("/opt/trn_rl_repo",):
    if _p not in sys.path:
        sys.path.insert(0, _p)

import ml_dtypes

import concourse.bass as bass
import concourse.mybir as mybir
import concourse.tile as tile
from concourse import bacc
from concourse.bass_utils import run_bass_kernel_spmd
from concourse.masks import make_upper_triangular

BF16 = mybir.dt.bfloat16
F32 = mybir.dt.float32

B, S, D, H, HD = 2, 2048, 1024, 16, 64
NCORES = 8
HPC = 4          # heads per core
DHC = HPC * HD   # 256: d-slice per core
P = 128
SB = S // P      # 16 seq blocks
KC = D // P      # 8 contraction chunks for projections
QG = 512         # q column group width
NQG = S // QG    # 4


def _build_body(ctx, tc, io):
    nc = tc.nc
    xT, wq, wk, wv, wo, out = (
        io["xT"], io["wq"], io["wk"], io["wv"], io["wo"], io["out"],
    )

    consts = ctx.enter_context(tc.tile_pool(name="consts", bufs=1))
    persist = ctx.enter_context(tc.tile_pool(name="persist", bufs=1))
    spool = ctx.enter_context(tc.tile_pool(name="spsum", bufs=2, space="PSUM"))
    cxpool = ctx.enter_context(tc.tile_pool(name="cxpsum", bufs=4, space="PSUM"))
    espool = ctx.enter_context(tc.tile_pool(name="es", bufs=6))
    nrmpool = ctx.enter_context(tc.tile_pool(name="nrm", bufs=4))
    outpool = ctx.enter_context(tc.tile_pool(name="outsb", bufs=3))
    drampool = ctx.enter_context(tc.tile_pool(name="dram", bufs=1, space="DRAM"))

    # DRAM bounce buffer for partition-broadcasting softmax reciprocals
    rsc = drampool.tile([16, QG], F32, tag="rsc", name="rsc")

    # triangular keep-mask for diagonal blocks: tri[i, j] = 1.0 iff j >= i
    tri = consts.tile([P, P], BF16, tag="tri", name="tri")
    make_upper_triangular(nc, tri[:], val=1.0, diag=True)

    # ---- load inputs ----
    xt = []
    for k in range(KC):
        t = persist.tile([P, S], BF16, tag=f"xt{k}", name=f"xt{k}")
        nc.sync.dma_start(out=t[:], in_=xT[k * P:(k + 1) * P, :])
        xt.append(t)
    wq_sb, wk_sb, wv_sb = [], [], []
    for name, dram, lst in (("wq", wq, wq_sb), ("wk", wk, wk_sb), ("wv", wv, wv_sb)):
        for k in range(KC):
            t = persist.tile([P, DHC], BF16, tag=f"{name}{k}", name=f"{name}{k}")
            nc.sync.dma_start(out=t[:], in_=dram[k * P:(k + 1) * P, :])
            lst.append(t)
    wo_sb = []
    for k in range(2):
        t = persist.tile([P, D], BF16, tag=f"wo{k}", name=f"wo{k}")
        nc.sync.dma_start(out=t[:], in_=wo[k * P:(k + 1) * P, :])
        wo_sb.append(t)

    # ---- projections ----
    # qT/kT: [256, 2048] bf16, partition chunk i holds heads (2i, 2i+1)
    qt = [persist.tile([P, S], BF16, tag=f"qt{i}", name=f"qt{i}") for i in range(2)]
    kt = [persist.tile([P, S], BF16, tag=f"kt{i}", name=f"kt{i}") for i in range(2)]
    for i in range(2):
        for j in range(NQG):
            ps = spool.tile([P, 2, QG], F32, tag="sp", name="sp")
            for which, (w_sb, dst) in enumerate(((wq_sb, qt), (wk_sb, kt))):
                for k in range(KC):
                    nc.tensor.matmul(
                        ps[:, which, :],
                        lhsT=w_sb[k][:, i * P:(i + 1) * P],
                        rhs=xt[k][:, j * QG:(j + 1) * QG],
                        start=(k == 0),
                        stop=(k == KC - 1),
                    )
                nc.vector.tensor_copy(dst[i][:, j * QG:(j + 1) * QG], ps[:, which, :])

    # v: natural layout per seq block: [128, 4 heads, 65] (col 64 = 1.0)
    v_sb = [persist.tile([P, HPC, HD + 1], BF16, tag=f"v{s}", name=f"v{s}") for s in range(SB)]
    for sv in range(SB // 2):
        ps = spool.tile([P, 2, QG], F32, tag="sp", name="sp")
        for par in range(2):
            s = 2 * sv + par
            for k in range(KC):
                nc.tensor.matmul(
                    ps[:, par, 0:DHC],
                    lhsT=xt[k][:, s * P:(s + 1) * P],
                    rhs=wv_sb[k][:],
                    start=(k == 0),
                    stop=(k == KC - 1),
                )
            src = ps[:, par, 0:DHC].rearrange("p (h d) -> p h d", h=HPC)
            nc.vector.tensor_copy(v_sb[s][:, :, 0:HD], src)
            nc.vector.memset(v_sb[s][:, :, HD:HD + 1], 1.0)

    # ---- attention ----
    # ctxT: normalized context, transposed: [256, 2048] bf16 (chunk i = heads 2i,2i+1)
    ctxT = [persist.tile([P, S], BF16, tag=f"ctxT{i}", name=f"ctxT{i}") for i in range(2)]
    for pair in range(2):           # heads (2*pair, 2*pair + 1)
        for g in range(NQG):
            cxs = [cxpool.tile([HD + 1, QG], F32, tag="cx", name="cx") for _ in range(2)]
            nkb = 4 * g + 4
            for kb in range(nkb):
                c0 = P * (kb - 4 * g) if kb >= 4 * g else 0
                sp_t = spool.tile([P, 2, QG], F32, tag="sp", name="sp")
                for hh in range(2):
                    nc.tensor.matmul(
                        sp_t[:, hh, c0:QG],
                        lhsT=kt[pair][hh * HD:(hh + 1) * HD, kb * P:(kb + 1) * P],
                        rhs=qt[pair][hh * HD:(hh + 1) * HD, g * QG + c0:(g + 1) * QG],
                        start=True,
                        stop=True,
                    )
                es_t = espool.tile([P, 2, QG], BF16, tag="es", name="es")
                nc.scalar.activation(
                    es_t[:, :, c0:QG], sp_t[:, :, c0:QG],
                    mybir.ActivationFunctionType.Exp, scale=0.125,
                )
                if kb >= 4 * g:
                    # triangular mask on the diagonal block, both heads at once
                    dst = es_t[:, :, c0:c0 + P]
                    t_ap = tri[:]
                    tri_b = bass.AP(t_ap.tensor, t_ap.offset,
                                    [t_ap.ap[0], [0, 2], t_ap.ap[1]])
                    nc.vector.tensor_mul(dst, dst, tri_b)
                for hh in range(2):
                    h = 2 * pair + hh
                    nc.tensor.matmul(
                        cxs[hh][:, c0:QG],
                        lhsT=v_sb[kb][:, h, :],
                        rhs=es_t[:, hh, c0:QG],
                        start=(kb == 0),
                        stop=(kb == nkb - 1),
                    )
            for hh in range(2):
                rc = nrmpool.tile([1, QG], F32, tag="rc", name="rc")
                nc.vector.reciprocal(rc[:], cxs[hh][HD:HD + 1, :])
                slot = (pair * NQG + g) * 2 + hh
                nc.sync.dma_start(out=rsc[slot:slot + 1, :], in_=rc[:])
                rb = nrmpool.tile([HD, QG], F32, tag="rb", name="rb")
                sl_ap = rsc[slot:slot + 1, :]
                rc_b = bass.AP(sl_ap.tensor, sl_ap.offset,
                               [[0, HD]] + list(sl_ap.ap[1:]))
                nc.sync.dma_start(out=rb[:], in_=rc_b)
                nc.vector.tensor_mul(
                    ctxT[pair][hh * HD:(hh + 1) * HD, g * QG:(g + 1) * QG],
                    cxs[hh][0:HD, :],
                    rb[:],
                )

    # ---- out-projection partial: out[m*128:...] = ctx @ Wo (rows 256g..) ----
    for m in range(SB):
        ps = spool.tile([P, 2, QG], F32, tag="sp", name="sp")
        for n2 in range(2):
            for kc in range(2):
                nc.tensor.matmul(
                    ps[:, n2, :],
                    lhsT=ctxT[kc][:, m * P:(m + 1) * P],
                    rhs=wo_sb[kc][:, n2 * QG:(n2 + 1) * QG],
                    start=(kc == 0),
                    stop=(kc == 1),
                )
        ot = outpool.tile([P, D], F32, tag="ot", name="ot")
        nc.scalar.copy(ot[:, 0:QG], ps[:, 0, :])
        nc.vector.tensor_copy(ot[:, QG:D], ps[:, 1, :])
        nc.sync.dma_start(out=out[m * P:(m + 1) * P, :], in_=ot[:])


def build_nc():
    from contextlib import ExitStack

    nc = bacc.Bacc()
    _ = ExitStack
    io = {
        "xT": nc.dram_tensor("xT", [D, S], BF16, kind="ExternalInput").ap(),
        "wq": nc.dram_tensor("wq", [D, DHC], BF16, kind="ExternalInput").ap(),
        "wk": nc.dram_tensor("wk", [D, DHC], BF16, kind="ExternalInput").ap(),
        "wv": nc.dram_tensor("wv", [D, DHC], BF16, kind="ExternalInput").ap(),
        "wo": nc.dram_tensor("wo", [DHC, D], BF16, kind="ExternalInput").ap(),
        "out": nc.dram_tensor("out", [S, D], F32, kind="ExternalOutput").ap(),
    }
    with tile.TileContext(nc) as tc:
        with ExitStack() as ctx:
            _build_body(ctx, tc, io)
    nc.finalize()
    return nc


_NC = None


def _get_nc():
    global _NC
    if _NC is None:
        _NC = build_nc()
    return _NC


def make_in_maps(x, Wq, Wk, Wv, Wo):
    bf = ml_dtypes.bfloat16
    x = np.asarray(x, dtype=np.float32)
    in_maps = []
    xTs = [np.ascontiguousarray(x[b].T).astype(bf) for b in range(B)]
    for c in range(NCORES):
        b, g = divmod(c, 4)
        sl = slice(DHC * g, DHC * (g + 1))
        in_maps.append({
            "xT": xTs[b],
            "wq": np.ascontiguousarray(np.asarray(Wq, np.float32)[:, sl]).astype(bf),
            "wk": np.ascontiguousarray(np.asarray(Wk, np.float32)[:, sl]).astype(bf),
            "wv": np.ascontiguousarray(np.asarray(Wv, np.float32)[:, sl]).astype(bf),
            "wo": np.ascontiguousarray(np.asarray(Wo, np.float32)[sl, :]).astype(bf),
        })
    return in_maps


def run(in_maps, trace=False, **kw):
    return run_bass_kernel_spmd(_get_nc(), in_maps, list(range(NCORES)),
                                trace=trace, **kw)


def kernel(x, Wq, Wk, Wv, Wo, bo):
    res = run(make_in_maps(x, Wq, Wk, Wv, Wo)).results
    bo = np.asarray(bo, np.float32)
    out = np.empty((B, S, D), np.float32)
    for b in range(B):
        acc = res[4 * b]["out"].astype(np.float32)
        for g in range(1, 4):
            acc = acc + res[4 * b + g]["out"]
        out[b] = acc + bo[None, :]
    return out


# revision 15
# speedup vs baseline: 1.1659x; 1.1659x over previous
"""Multi-head causal attention (B=2, S=2048, D=1024, H=16, hd=64) on 8 trn2 cores.

Sharding: core c handles batch b = c//4 and head-group g = c%4 (heads 4g..4g+4,
d-slice 256g..256g+256 of the QKV projections / Wo rows).  Each core computes a
partial out-projection [2048, 1024]; the host sums the 4 head-group partials per
batch and adds the bias.

Per-core kernel (all matmuls bf16, accumulate f32 in PSUM):
  qT/kT = (x @ Wq/k)^T computed directly as [256, 2048] via lhsT=W chunks.
  v     = x @ Wv in natural [seq, head, 64(+1 ones col)] layout (ones column
          makes the attention rowsum fall out of the ctx matmul).
  S^T   = k_h @ q_h^T  [kpos, qpos] tiles; exp via ACT (scale=1/8) PSUM->SBUF;
          causal handled by skipping invalid column blocks + one triangular
          bf16 mask multiply on diagonal blocks.
  ctx~T = v'_h^T @ expS^T accumulated over kpos blocks -> [65, 512] PSUM
          (row 64 = softmax denominator).
  out  += (ctx~T / rowsum)^T @ Wo rows  (normalization via DVE multiply with a
          GPSIMD partition-broadcast reciprocal).
"""

import sys

import numpy as np

for _p in ("/opt/trn_rl_repo",):
    if _p not in sys.path:
        sys.path.insert(0, _p)

import ml_dtypes

import concourse.bass as bass
import concourse.mybir as mybir
import concourse.tile as tile
from concourse import bacc
from concourse.bass_utils import run_bass_kernel_spmd
from concourse.masks import make_upper_triangular

BF16 = mybir.dt.bfloat16
F32 = mybir.dt.float32

B, S, D, H, HD = 2, 2048, 1024, 16, 64
NCORES = 8
HPC = 4          # heads per core
DHC = HPC * HD   # 256: d-slice per core
P = 128
SB = S // P      # 16 seq blocks
KC = D // P      # 8 contraction chunks for projections
QG = 512         # q column group width
NQG = S // QG    # 4


def _build_body(ctx, tc, io):
    nc = tc.nc
    xT, wq, wk, wv, wo, out = (
        io["xT"], io["wq"], io["wk"], io["wv"], io["wo"], io["out"],
    )

    consts = ctx.enter_context(tc.tile_pool(name="consts", bufs=1))
    persist = ctx.enter_context(tc.tile_pool(name="persist", bufs=1))
    spool = ctx.enter_context(tc.tile_pool(name="spsum", bufs=2, space="PSUM"))
    cxpool = ctx.enter_context(tc.tile_pool(name="cxpsum", bufs=4, space="PSUM"))
    espool = ctx.enter_context(tc.tile_pool(name="es", bufs=6))
    nrmpool = ctx.enter_context(tc.tile_pool(name="nrm", bufs=4))
    outpool = ctx.enter_context(tc.tile_pool(name="outsb", bufs=3))
    drampool = ctx.enter_context(tc.tile_pool(name="dram", bufs=1, space="DRAM"))

    # DRAM bounce buffer for partition-broadcasting softmax reciprocals
    rsc = drampool.tile([16, QG], F32, tag="rsc", name="rsc")

    # triangular keep-mask for diagonal blocks: tri[i, j] = 1.0 iff j >= i
    tri = consts.tile([P, P], BF16, tag="tri", name="tri")
    make_upper_triangular(nc, tri[:], val=1.0, diag=True)

    # ---- load inputs (weights interleaved with x chunks so the first
    # projection matmuls can start as soon as chunk 0 has landed) ----
    xt, wq_sb, wk_sb, wv_sb = [], [], [], []
    named = (("wq", wq, wq_sb), ("wk", wk, wk_sb), ("wv", wv, wv_sb))
    for k in range(KC):
        for name, dram, lst in named:
            t = persist.tile([P, DHC], BF16, tag=f"{name}{k}", name=f"{name}{k}")
            nc.sync.dma_start(out=t[:], in_=dram[k * P:(k + 1) * P, :])
            lst.append(t)
        t = persist.tile([P, S], BF16, tag=f"xt{k}", name=f"xt{k}")
        nc.sync.dma_start(out=t[:], in_=xT[k * P:(k + 1) * P, :])
        xt.append(t)
    wo_sb = []
    for k in range(2):
        t = persist.tile([P, D], BF16, tag=f"wo{k}", name=f"wo{k}")
        nc.sync.dma_start(out=t[:], in_=wo[k * P:(k + 1) * P, :])
        wo_sb.append(t)

    # ---- projections + attention, interleaved per head-pair ----
    # v first (natural layout per seq block: [128, 4 heads, 65], col 64 = 1.0)
    v_sb = [persist.tile([P, HPC, HD + 1], BF16, tag=f"v{s}", name=f"v{s}") for s in range(SB)]
    for sv in range(SB // 2):
        ps = spool.tile([P, 2, QG], F32, tag="sp", name="sp")
        for par in range(2):
            s = 2 * sv + par
            for k in range(KC):
                nc.tensor.matmul(
                    ps[:, par, 0:DHC],
                    lhsT=xt[k][:, s * P:(s + 1) * P],
                    rhs=wv_sb[k][:],
                    start=(k == 0),
                    stop=(k == KC - 1),
                )
            src_ap = ps[:, par, 0:DHC].rearrange("p (h d) -> p h d", h=HPC)
            nc.vector.tensor_copy(v_sb[s][:, :, 0:HD], src_ap)
            nc.vector.memset(v_sb[s][:, :, HD:HD + 1], 1.0)

    # qT/kT: [256, 2048] bf16, partition chunk i holds heads (2i, 2i+1)
    qt = [persist.tile([P, S], BF16, tag=f"qt{i}", name=f"qt{i}") for i in range(2)]
    kt = [persist.tile([P, S], BF16, tag=f"kt{i}", name=f"kt{i}") for i in range(2)]
    # ctxT: normalized context, transposed: [256, 2048] bf16 (chunk i = heads 2i,2i+1)
    ctxT = [persist.tile([P, S], BF16, tag=f"ctxT{i}", name=f"ctxT{i}") for i in range(2)]

    for pair in range(2):           # heads (2*pair, 2*pair + 1)
        # -- projections for this pair's d-chunk --
        for j in range(NQG):
            ps = spool.tile([P, 2, QG], F32, tag="sp", name="sp")
            for which, (w_sb, dst) in enumerate(((wq_sb, qt), (wk_sb, kt))):
                for k in range(KC):
                    nc.tensor.matmul(
                        ps[:, which, :],
                        lhsT=w_sb[k][:, pair * P:(pair + 1) * P],
                        rhs=xt[k][:, j * QG:(j + 1) * QG],
                        start=(k == 0),
                        stop=(k == KC - 1),
                    )
                nc.vector.tensor_copy(dst[pair][:, j * QG:(j + 1) * QG], ps[:, which, :])

        # -- attention for this pair --
        for g in range(NQG):
            cxs = [cxpool.tile([HD + 1, QG], F32, tag="cx", name="cx") for _ in range(2)]
            nkb = 4 * g + 4
            for kb in range(nkb):
                c0 = P * (kb - 4 * g) if kb >= 4 * g else 0
                sp_t = spool.tile([P, 2, QG], F32, tag="sp", name="sp")
                for hh in range(2):
                    nc.tensor.matmul(
                        sp_t[:, hh, c0:QG],
                        lhsT=kt[pair][hh * HD:(hh + 1) * HD, kb * P:(kb + 1) * P],
                        rhs=qt[pair][hh * HD:(hh + 1) * HD, g * QG + c0:(g + 1) * QG],
                        start=True,
                        stop=True,
                    )
                es_t = espool.tile([P, 2, QG], BF16, tag="es", name="es")
                nc.scalar.activation(
                    es_t[:, :, c0:QG], sp_t[:, :, c0:QG],
                    mybir.ActivationFunctionType.Exp, scale=0.125,
                )
                if kb >= 4 * g:
                    # triangular mask on the diagonal block, both heads at once
                    dst = es_t[:, :, c0:c0 + P]
                    t_ap = tri[:]
                    tri_b = bass.AP(t_ap.tensor, t_ap.offset,
                                    [t_ap.ap[0], [0, 2], t_ap.ap[1]])
                    nc.vector.tensor_mul(dst, dst, tri_b)
                for hh in range(2):
                    h = 2 * pair + hh
                    nc.tensor.matmul(
                        cxs[hh][:, c0:QG],
                        lhsT=v_sb[kb][:, h, :],
                        rhs=es_t[:, hh, c0:QG],
                        start=(kb == 0),
                        stop=(kb == nkb - 1),
                    )
            for hh in range(2):
                rc = nrmpool.tile([1, QG], F32, tag="rc", name="rc")
                nc.vector.reciprocal_approx_fast(rc[:], cxs[hh][HD:HD + 1, :])
                slot = (pair * NQG + g) * 2 + hh
                nc.sync.dma_start(out=rsc[slot:slot + 1, :], in_=rc[:])
                rb = nrmpool.tile([HD, QG], F32, tag="rb", name="rb")
                sl_ap = rsc[slot:slot + 1, :]
                rc_b = bass.AP(sl_ap.tensor, sl_ap.offset,
                               [[0, HD]] + list(sl_ap.ap[1:]))
                nc.sync.dma_start(out=rb[:], in_=rc_b)
                nc.vector.tensor_mul(
                    ctxT[pair][hh * HD:(hh + 1) * HD, g * QG:(g + 1) * QG],
                    cxs[hh][0:HD, :],
                    rb[:],
                )

    # ---- out-projection partial: out[m*128:...] = ctx @ Wo (rows 256g..) ----
    for m in range(SB):
        ps = spool.tile([P, 2, QG], F32, tag="sp", name="sp")
        for n2 in range(2):
            for kc in range(2):
                nc.tensor.matmul(
                    ps[:, n2, :],
                    lhsT=ctxT[kc][:, m * P:(m + 1) * P],
                    rhs=wo_sb[kc][:, n2 * QG:(n2 + 1) * QG],
                    start=(kc == 0),
                    stop=(kc == 1),
                )
        ot = outpool.tile([P, D], F32, tag="ot", name="ot")
        nc.scalar.copy(ot[:, 0:QG], ps[:, 0, :])
        nc.vector.tensor_copy(ot[:, QG:D], ps[:, 1, :])
        nc.sync.dma_start(out=out[m * P:(m + 1) * P, :], in_=ot[:])


def build_nc():
    from contextlib import ExitStack

    nc = bacc.Bacc()
    _ = ExitStack
    io = {
        "xT": nc.dram_tensor("xT", [D, S], BF16, kind="ExternalInput").ap(),
        "wq": nc.dram_tensor("wq", [D, DHC], BF16, kind="ExternalInput").ap(),
        "wk": nc.dram_tensor("wk", [D, DHC], BF16, kind="ExternalInput").ap(),
        "wv": nc.dram_tensor("wv", [D, DHC], BF16, kind="ExternalInput").ap(),
        "wo": nc.dram_tensor("wo", [DHC, D], BF16, kind="ExternalInput").ap(),
        "out": nc.dram_tensor("out", [S, D], F32, kind="ExternalOutput").ap(),
    }
    with tile.TileContext(nc) as tc:
        with ExitStack() as ctx:
            _build_body(ctx, tc, io)
    nc.finalize()
    return nc


_NC = None


def _get_nc():
    global _NC
    if _NC is None:
        _NC = build_nc()
    return _NC


def make_in_maps(x, Wq, Wk, Wv, Wo):
    bf = ml_dtypes.bfloat16
    x = np.asarray(x, dtype=np.float32)
    in_maps = []
    xTs = [np.ascontiguousarray(x[b].T).astype(bf) for b in range(B)]
    for c in range(NCORES):
        b, g = divmod(c, 4)
        sl = slice(DHC * g, DHC * (g + 1))
        in_maps.append({
            "xT": xTs[b],
            "wq": np.ascontiguousarray(np.asarray(Wq, np.float32)[:, sl]).astype(bf),
            "wk": np.ascontiguousarray(np.asarray(Wk, np.float32)[:, sl]).astype(bf),
            "wv": np.ascontiguousarray(np.asarray(Wv, np.float32)[:, sl]).astype(bf),
            "wo": np.ascontiguousarray(np.asarray(Wo, np.float32)[sl, :]).astype(bf),
        })
    return in_maps


def run(in_maps, trace=False, **kw):
    return run_bass_kernel_spmd(_get_nc(), in_maps, list(range(NCORES)),
                                trace=trace, **kw)


def kernel(x, Wq, Wk, Wv, Wo, bo):
    res = run(make_in_maps(x, Wq, Wk, Wv, Wo)).results
    bo = np.asarray(bo, np.float32)
    out = np.empty((B, S, D), np.float32)
    for b in range(B):
        acc = res[4 * b]["out"].astype(np.float32)
        for g in range(1, 4):
            acc = acc + res[4 * b + g]["out"]
        out[b] = acc + bo[None, :]
    return out
